# revision 41
# baseline (speedup 1.0000x reference)
"""ANI-style per-species MLP (384->160->128->96->1, CELU) over [B=128, A=512]
atoms with species routing, atom-summed to [B]. 8-core SPMD Trainium2 kernel.

v2: atom-parallel sharding as before (species-grouped atoms dealt round-robin
to 8 cores, zero-AEV dummy padding corrected on host), with:
  - AEV streamed as fp8e4m3 (rhs) against bf16 weights (lhsT): 4x less DMA.
  - bf16 activations in SBUF; f32 PSUM accumulate.
  - celu split: V1 = exp(ACT) + clamp(DVE ts, bf16 4x mode) + stt(DVE);
    V2 (layer1) = exp+relu on ACT + clamp + tensor_add(DVE 2x) to balance
    ACT vs DVE occupancy.
  - L0's 32-wide output chunk for the 4 tiles of a batch lands in ONE psum
    bank at partition offsets 32j -> one merged celu instead of 4.
  - software-pipelined emission B(k) -> A(k+1) -> C(k) so the PE queue never
    waits on celu latency; PSUM pools shared (4 + 3 + 1 banks).
"""

import os
import sys

import numpy as np

try:
    import concourse  # noqa: F401
except ImportError:
    sys.path.insert(0, "/opt/trn_rl_repo")

N_CORES = 8
B, A, FEAT = 128, 512, 384
N_SPECIES = 4
H0, H1, H2 = 160, 128, 96
ALPHA = 0.1
LNA = float(np.log(ALPHA))

WPS = 3 * 160 + 128 + 96 + 1  # 705 weight-pack columns per species
CPS = 6                       # constant-pack columns per species

X_FP8 = bool(int(os.environ.get("BASSNN_X_FP8", "1")))
L0_DP = bool(int(os.environ.get("BASSNN_L0_DP", "1")))

TRACE = bool(int(os.environ.get("BASSNN_TRACE", "0")))
LAST = {}

_progs = {}


def _maybe_register_ntff_hook():
    try:
        import types

        import antenv
        from antenv import axon_hooks  # noqa: F401
        return
    except ImportError:
        pass
    try:
        import types

        import antenv
        from trn_agent_boot.trn_boot import _ntff_profile_via_ctypes

        mod = types.ModuleType("antenv.axon_hooks")
        holder = [None]
        mod.set_axon_ntff_profile_hook = lambda h: holder.__setitem__(0, h)
        mod.get_axon_ntff_profile_hook = lambda: holder[0]
        sys.modules["antenv.axon_hooks"] = mod
        antenv.axon_hooks = mod
        mod.set_axon_ntff_profile_hook(
            _ntff_profile_via_ctypes("/opt/axon/libaxon_pjrt.so")
        )
    except Exception:
        pass


def _tiles_for_groups(G):
    """Per-species padded group sizes -> list of (species, slot0, n_atoms)."""
    tiles = []
    slot0 = 0
    for s, g in enumerate(G):
        a = 0
        while a < g:
            na = 4 if g - a >= 4 else g - a
            tiles.append((s, slot0 + a, na))
            a += na
        slot0 += g
    return tiles


def _batches_for_tiles(tiles):
    bs = [list(range(b, min(b + 4, len(tiles)))) for b in range(0, len(tiles), 4)]
    # A short trailing batch costs double at the end of the pipeline: the
    # cycle before it gets few interleave thunks AND its own serial celu
    # chain is fully exposed. Slot it second instead, where its chain hides
    # behind a full batch's interleave (position 0 would starve the
    # prologue, measured earlier).
    if len(bs) > 2 and len(bs[-1]) < 4:
        bs = [bs[0], bs[-1]] + bs[1:-1]
    return bs


def _build_program(G, S):
    import concourse.bass as bass  # noqa: F401
    import concourse.tile as tile
    from concourse import bacc, mybir

    F32 = mybir.dt.float32
    BF16 = mybir.dt.bfloat16
    X8 = mybir.dt.float8e4 if X_FP8 else BF16
    EXP = mybir.ActivationFunctionType.Exp
    RELU = mybir.ActivationFunctionType.Relu
    MIN = mybir.AluOpType.min
    MAX = mybir.AluOpType.max
    ADD = mybir.AluOpType.add
    SUB = mybir.AluOpType.subtract

    tiles = _tiles_for_groups(G)
    ntiles = len(tiles)
    batches = _batches_for_tiles(tiles)
    nbatches = len(batches)
    WB0 = WPS * N_SPECIES
    CB0 = CPS * N_SPECIES
    WCOLS = WB0 + 128 * nbatches
    CCOLS = CB0 + 2 * nbatches
    # batch k covers consecutive slots [bslot0[k], bslot0[k] + bna[k])
    bslot0 = [tiles[bt[0]][1] for bt in batches]
    bna = [sum(tiles[ti][2] for ti in bt) for bt in batches]
    XB = max(bna) * 384

    nc = bacc.Bacc("TRN2", target_bir_lowering=False, debug=False,
                   num_devices=N_CORES)
    xt = nc.dram_tensor("xt", [128, S * 384], X8, kind="ExternalInput").ap()
    wp = nc.dram_tensor("wp", [128, WCOLS], BF16, kind="ExternalInput").ap()
    cp = nc.dram_tensor("cp", [128, CCOLS], F32, kind="ExternalInput").ap()
    yo = nc.dram_tensor("yo", [1, 512], F32, kind="ExternalOutput").ap()

    with tile.TileContext(nc) as tc:
        with (
            tc.tile_pool(name="wcpool", bufs=1) as wcpool,
            tc.tile_pool(name="xpool", bufs=3) as xpool,
            tc.tile_pool(name="ewpool", bufs=6) as ewpool,
            tc.tile_pool(name="ypool", bufs=6) as ypool,
            tc.tile_pool(name="ppa", bufs=2, space="PSUM") as ppa,
            tc.tile_pool(name="pp1", bufs=2, space="PSUM") as pp1,
            tc.tile_pool(name="ppb0", bufs=1, space="PSUM") as ppb0,
            tc.tile_pool(name="ppb2", bufs=2, space="PSUM") as ppb2,
            tc.tile_pool(name="pp3", bufs=1, space="PSUM") as pp3,
        ):
            # weights/constants on the gpsimd DMA queue, x batches on sync:
            # descriptor generation and transfers overlap. The first species'
            # L0 weight block and the constants go first so the opening
            # matmuls and celus are not gated on the full pack (subtile deps).
            w = wcpool.tile([128, WCOLS], BF16, name="w")
            c = wcpool.tile([128, CCOLS], F32, name="c")
            nc.gpsimd.dma_start(c[:], cp[:])
            s_first = tiles[batches[0][0]][0]
            wA, wB = WPS * s_first, WPS * s_first + WPS
            nc.gpsimd.dma_start(w[:, wA: wA + 480], wp[:, wA: wA + 480])
            nc.gpsimd.dma_start(w[:, wA + 480: wB], wp[:, wA + 480: wB])
            if wA > 0:
                nc.gpsimd.dma_start(w[:, 0:wA], wp[:, 0:wA])
            nc.gpsimd.dma_start(w[:, wB:WCOLS], wp[:, wB:WCOLS])

            def wcol(s, off, n):
                return w[:, s * WPS + off: s * WPS + off + n]

            def ccol(s, k, parts):
                return c[0:parts, s * CPS + k: s * CPS + k + 1]

            p3 = pp3.tile([1, 512], F32)
            # L3 partials accumulate with start=False throughout (the first
            # emitted tile may be a short batch covering only part of the
            # 512 columns), so zero the bank explicitly once.
            nc.vector.memzero(p3[:])
            n3 = [0]

            def celu_v1(y_ap, p_ap, e_ap, t_ap, ebias, mbias):
                # y = (P max mbias) + min(alpha*e^(10P+ebias) - alpha, 0)
                nc.scalar.activation(e_ap, p_ap, EXP, bias=ebias, scale=10.0)
                nc.vector.tensor_scalar(t_ap, e_ap, ALPHA, 0.0, SUB, MIN)
                nc.vector.scalar_tensor_tensor(y_ap, p_ap, mbias, t_ap,
                                               MAX, ADD)

            def celu_v2(y_ap, p_ap, e_ap, t_ap, r_ap, ebias, rbias):
                # y = relu(P + rbias) + min(alpha*e^(10P+ebias) - alpha, 0)
                nc.scalar.activation(e_ap, p_ap, EXP, bias=ebias, scale=10.0)
                nc.scalar.activation(r_ap, p_ap, RELU, bias=rbias, scale=1.0)
                nc.vector.tensor_scalar(t_ap, e_ap, ALPHA, 0.0, SUB, MIN)
                nc.vector.tensor_add(y_ap, r_ap, t_ap)

            xts = {}

            def dma_batch(k, split_first=False):
                t = xpool.tile([128, XB], X8, name="xts")
                o = bslot0[k] * 384
                if split_first:
                    n0 = tiles[batches[k][0]][2] * 384
                    nc.sync.dma_start(t[:, 0:n0], xt[:, o: o + n0])
                    if n0 < bna[k] * 384:
                        nc.sync.dma_start(t[:, n0: bna[k] * 384],
                                          xt[:, o + n0: o + bna[k] * 384])
                else:
                    nc.sync.dma_start(t[:, 0: bna[k] * 384],
                                      xt[:, o: o + bna[k] * 384])
                xts[k] = t

            y0as = {}
            y0bs = {}
            y1s = {}
            y2s = {}
            p0bs = {}

            def l0_mm_thunks(k):
                """Batch k's L0 matmuls as 12 thunks (2 mms each)."""
                batch = batches[k]
                xv = xts[k].rearrange("p (a f m) -> p a f m",
                                      a=XB // 384, f=3, m=128)
                p0b = ppb0.tile([128, 512], F32, tag="pb", name="p0b")
                p0bs[k] = p0b
                p0as = {}
                thunks = []
                for j, ti in enumerate(batch):
                    s, a0, na = tiles[ti]
                    N = na * 128
                    aoff = a0 - bslot0[k]
                    p0a = ppa.tile([128, 512], F32, tag="pa", name="p0a")
                    p0as[ti] = p0a

                    pm = (mybir.MatmulPerfMode.DoublePixel
                          if (L0_DP and X_FP8) else None)

                    def mk(j=j, s=s, na=na, N=N, aoff=aoff, p0a=p0a,
                           pm=pm, fc=0):
                        rhs = xv[:, aoff: aoff + na, fc, :]
                        nc.tensor.matmul(p0a[:, 0:N],
                                         wcol(s, fc * 160, 128),
                                         rhs, start=(fc == 0), stop=(fc == 2),
                                         perf_mode=pm)
                        nc.tensor.matmul(p0b[32 * j: 32 * j + 32, 0:N],
                                         wcol(s, fc * 160 + 128, 32),
                                         rhs, start=(fc == 0), stop=(fc == 2),
                                         tile_position=(0, 32 * j),
                                         perf_mode=pm)
                    for fc in range(3):
                        thunks.append(
                            lambda mk=mk, fc=fc: mk(fc=fc))
                return thunks, p0as

            def l0a_celu(k, j, p0as):
                ti = batches[k][j]
                s, a0, na = tiles[ti]
                N = na * 128
                ea = ewpool.tile([128, 512], BF16, name="ea")
                ta = ewpool.tile([128, 512], BF16, name="ta")
                y0a = ypool.tile([128, 512], BF16, name="y0a")
                celu_v1(y0a[:, 0:N], p0as[ti][:, 0:N], ea[:, 0:N],
                        ta[:, 0:N], ccol(s, 0, 128), ccol(s, 1, 128))
                y0as[ti] = y0a

            def l0b_celu(k):
                L = len(batches[k])
                em = ewpool.tile([128, 512], BF16, name="em")
                tm = ewpool.tile([128, 512], BF16, name="tm")
                y0b = ypool.tile([128, 512], BF16, name="y0b", bufs=2)
                P = 32 * L
                celu_v1(y0b[0:P, :], p0bs[k][0:P, :], em[0:P, :], tm[0:P, :],
                        c[0:P, CB0 + 2 * k: CB0 + 2 * k + 1],
                        c[0:P, CB0 + 2 * k + 1: CB0 + 2 * k + 2])
                y0bs[k] = y0b

            def steady(k):
                """B(k) with A(k+1)'s L0 matmuls interleaved, then C(k)."""
                batch = batches[k]
                y0b = y0bs[k]
                if k + 2 < nbatches:
                    dma_batch(k + 2)
                p1s = {}
                for j, ti in enumerate(batch):
                    s, a0, na = tiles[ti]
                    N = na * 128
                    p1 = pp1.tile([128, 512], F32, tag="p1", name="p1")
                    nc.tensor.matmul(p1[:, 0:N], wcol(s, 480, 128),
                                     y0as[ti][:, 0:N], start=True, stop=False)
                    nc.tensor.matmul(
                        p1[:, 0:N],
                        w[32 * j: 32 * j + 32,
                          WB0 + 128 * k: WB0 + 128 * (k + 1)],
                        y0b[32 * j: 32 * j + 32, 0:N],
                        start=False, stop=True,
                        tile_position=(32 * j, 0))
                    p1s[ti] = p1
                for j, ti in enumerate(batch):
                    s, a0, na = tiles[ti]
                    N = na * 128
                    e1 = ewpool.tile([128, 512], BF16, name="e1")
                    t1 = ewpool.tile([128, 512], BF16, name="t1")
                    r1 = ewpool.tile([128, 512], BF16, name="r1")
                    y1 = ypool.tile([128, 512], BF16, name="y1", bufs=5)
                    celu_v2(y1[:, 0:N], p1s[ti][:, 0:N], e1[:, 0:N],
                            t1[:, 0:N], r1[:, 0:N],
                            ccol(s, 2, 128), ccol(s, 3, 128))
                    y1s[ti] = y1
                if k + 1 < nbatches:
                    thunks, p0as_next = l0_mm_thunks(k + 1)
                else:
                    thunks, p0as_next = [], None
                chunks = [thunks[3 * i: 3 * i + 3] for i in range(4)]
                nnext = len(batches[k + 1]) if p0as_next is not None else 0
                for j, ti in enumerate(batch):
                    for th in chunks[j]:
                        th()
                    if p0as_next is not None and j < nnext:
                        l0a_celu(k + 1, j, p0as_next)
                    if j == len(batch) - 1:
                        # all of next batch's L0 matmuls are in by now:
                        # finish its celus so the merged-L0b stt lands well
                        # before the next cycle's L1b matmuls need it.
                        for ch in chunks[len(batch):]:
                            for th in ch:
                                th()
                        if p0as_next is not None:
                            for jj in range(len(batch), nnext):
                                l0a_celu(k + 1, jj, p0as_next)
                            l0b_celu(k + 1)
                    s, a0, na = tiles[ti]
                    N = na * 128
                    p2 = ppb2.tile([96, 512], F32, tag="p2", name="p2")
                    nc.tensor.matmul(p2[:, 0:N], wcol(s, 608, 96),
                                     y1s[ti][:, 0:N], start=True, stop=True)
                    e2 = ewpool.tile([96, 512], BF16, name="e2")
                    t2 = ewpool.tile([96, 512], BF16, name="t2")
                    y2 = ypool.tile([96, 512], BF16, name="y2")
                    celu_v1(y2[:, 0:N], p2[:, 0:N], e2[:, 0:N], t2[:, 0:N],
                            ccol(s, 4, 96), ccol(s, 5, 96))
                    y2s[ti] = y2
                for j, ti in enumerate(batch):
                    s, a0, na = tiles[ti]
                    N = na * 128
                    nc.tensor.matmul(p3[0:1, 0:N],
                                     wcol(s, 704, 1)[0:96, :],
                                     y2s[ti][0:96, 0:N],
                                     start=False,
                                     stop=(n3[0] == ntiles - 1),
                                     skip_group_check=True)
                    n3[0] += 1

            dma_batch(0, split_first=True)
            if nbatches > 1:
                dma_batch(1)
            thunks0, p0as0 = l0_mm_thunks(0)
            for th in thunks0:
                th()
            for j in range(len(batches[0])):
                l0a_celu(0, j, p0as0)
            l0b_celu(0)
            for k in range(nbatches):
                steady(k)

            t3 = wcpool.tile([1, 512], F32, name="t3")
            nc.scalar.copy(t3[:], p3[:])
            nc.sync.dma_start(yo[:], t3[:])

    nc.compile()
    return nc


def _celu64(z):
    return np.where(z > 0, z, ALPHA * np.expm1(np.minimum(z, 0) / ALPHA))


def _bf16_round(x):
    import ml_dtypes
    return np.asarray(x, np.float32).astype(ml_dtypes.bfloat16).astype(np.float64)


def kernel(fullaev, species, W0, b0, W1, b1, W2, b2, W3, b3):
    import ml_dtypes
    from concourse import bass_utils, mybir

    fullaev = np.ascontiguousarray(np.asarray(fullaev, dtype=np.float32))
    species = np.asarray(species, dtype=np.int32)
    Ws = [np.asarray(w, dtype=np.float32) for w in (W0, W1, W2, W3)]
    bs = [np.asarray(b, dtype=np.float32) for b in (b0, b1, b2, b3)]

    # --- species grouping: per-core slot assignment ---------------------
    ids = [np.where(species == s)[0] for s in range(N_SPECIES)]
    n = [len(i) for i in ids]
    G = [-(-n[s] // N_CORES) if n[s] else 0 for s in range(N_SPECIES)]
    S = sum(G)
    key = (tuple(G), X_FP8, L0_DP)
    if key not in _progs:
        _progs[key] = _build_program(G, S)
    nc = _progs[key]

    tiles = _tiles_for_groups(G)
    batches = _batches_for_tiles(tiles)
    nbatches = len(batches)
    WB0 = WPS * N_SPECIES
    CB0 = CPS * N_SPECIES
    WCOLS = WB0 + 128 * nbatches
    CCOLS = CB0 + 2 * nbatches

    # --- fold constants (float64, with bf16-rounded weights) ------------
    cpack = np.zeros((128, CCOLS), np.float32)
    wpack = np.zeros((128, WCOLS), np.float32)
    c3 = np.zeros(N_SPECIES)
    K0 = np.zeros(N_SPECIES)
    c1s = {}
    for s in range(N_SPECIES):
        w1, w2, w3 = (_bf16_round(Ws[l][s]) for l in (1, 2, 3))
        bb0, bb1, bb2, bb3 = (b[s].astype(np.float64) for b in bs)
        c1 = bb1 + w1 @ bb0
        c1s[s] = c1
        c3[s] = bb3[0] + w3[0] @ bb2
        # device contribution of a dummy (zero-AEV) atom, excluding c3
        y0d = _celu64(bb0) - bb0
        y1d = _celu64(w1 @ y0d + c1)
        y2d = _celu64(w2 @ y1d + bb2) - bb2
        K0[s] = w3[0] @ y2d

        cb = s * CPS
        cpack[:, cb + 0] = 10.0 * bb0[:128] + LNA
        cpack[:, cb + 1] = -bb0[:128]
        cpack[:, cb + 2] = 10.0 * c1 + LNA
        cpack[:, cb + 3] = c1
        cpack[:96, cb + 4] = 10.0 * bb2 + LNA
        cpack[:96, cb + 5] = -bb2

        wb = s * WPS
        for fc in range(3):
            blk = Ws[0][s][:, fc * 128:(fc + 1) * 128].T  # [128in, 160out]
            wpack[:, wb + fc * 160: wb + fc * 160 + 160] = blk
        wpack[:, wb + 480: wb + 608] = Ws[1][s][:, :128].T
        wpack[:, wb + 608: wb + 704] = Ws[2][s].T
        wpack[:96, wb + 704] = Ws[3][s][0, :]

    for bi, batch in enumerate(batches):
        for j, ti in enumerate(batch):
            s = tiles[ti][0]
            b0b = bs[0][s].astype(np.float64)[128:]
            cpack[32 * j: 32 * j + 32, CB0 + 2 * bi] = 10.0 * b0b + LNA
            cpack[32 * j: 32 * j + 32, CB0 + 2 * bi + 1] = -b0b
            wpack[32 * j: 32 * j + 32,
                  WB0 + 128 * bi: WB0 + 128 * (bi + 1)] = Ws[1][s][:, 128:].T

    wpack_b = wpack.astype(ml_dtypes.bfloat16)
    x_np_dtype = mybir.dt.np(mybir.dt.float8e4 if X_FP8 else mybir.dt.bfloat16)

    # --- per-core transposed, species-sorted AEV blocks -----------------
    in_maps = []
    dummy_counts = np.zeros((N_CORES, N_SPECIES), np.int64)
    for cid in range(N_CORES):
        xtc = np.zeros((128, S, 3, 128), np.float32)
        slot0 = 0
        for s in range(N_SPECIES):
            mine = ids[s][cid::N_CORES]
            nr = len(mine)
            dummy_counts[cid, s] = G[s] - nr
            if nr:
                g = fullaev[:, mine, :]               # [128, nr, 384]
                t = g.transpose(2, 1, 0)              # [384, nr, 128]
                xtc[:, slot0: slot0 + nr, :, :] = (
                    t.reshape(3, 128, nr, 128).transpose(1, 2, 0, 3)
                )
            slot0 += G[s]
        xq = xtc.reshape(128, S * 384).astype(x_np_dtype)
        in_maps.append({"xt": xq, "wp": wpack_b, "cp": cpack})

    if TRACE:
        _maybe_register_ntff_hook()
    res = bass_utils.run_bass_kernel_spmd(
        nc, in_maps, core_ids=list(range(N_CORES)), trace=TRACE
    )
    LAST["exec_time_ns"] = res.exec_time_ns
    LAST["trace"] = res.instructions_and_trace[1] if res.instructions_and_trace else None

    out = np.zeros(128, np.float64)
    for cid in range(N_CORES):
        out += (res.results[cid]["yo"][0].astype(np.float64)
                .reshape(4, 128).sum(axis=0))
    for s in range(N_SPECIES):
        out += n[s] * c3[s] - dummy_counts[:, s].sum() * K0[s]
    return out.astype(np.float32)


# revision 42
# speedup vs baseline: 1.1919x; 1.1919x over previous
"""ANI-style per-species MLP (384->160->128->96->1, CELU) over [B=128, A=512]
atoms with species routing, atom-summed to [B]. 8-core SPMD Trainium2 kernel.

v2: atom-parallel sharding as before (species-grouped atoms dealt round-robin
to 8 cores, zero-AEV dummy padding corrected on host), with:
  - AEV streamed as fp8e4m3 (rhs) against bf16 weights (lhsT): 4x less DMA.
  - bf16 activations in SBUF; f32 PSUM accumulate.
  - celu split: V1 = exp(ACT) + clamp(DVE ts, bf16 4x mode) + stt(DVE);
    V2 (layer1) = exp+relu on ACT + clamp + tensor_add(DVE 2x) to balance
    ACT vs DVE occupancy.
  - L0's 32-wide output chunk for the 4 tiles of a batch lands in ONE psum
    bank at partition offsets 32j -> one merged celu instead of 4.
  - software-pipelined emission B(k) -> A(k+1) -> C(k) so the PE queue never
    waits on celu latency; PSUM pools shared (4 + 3 + 1 banks).
"""

import os
import sys

import numpy as np

try:
    import concourse  # noqa: F401
except ImportError:
    sys.path.insert(0, "/opt/trn_rl_repo")

N_CORES = 8
B, A, FEAT = 128, 512, 384
N_SPECIES = 4
H0, H1, H2 = 160, 128, 96
ALPHA = 0.1
LNA = float(np.log(ALPHA))

WPS = 3 * 160 + 128 + 96 + 1  # 705 weight-pack columns per species
CPS = 6                       # constant-pack columns per species

X_FP8 = bool(int(os.environ.get("BASSNN_X_FP8", "1")))
L0_DP = bool(int(os.environ.get("BASSNN_L0_DP", "1")))

TRACE = bool(int(os.environ.get("BASSNN_TRACE", "0")))
LAST = {}

_progs = {}


def _maybe_register_ntff_hook():
    try:
        import types

        import antenv
        from antenv import axon_hooks  # noqa: F401
        return
    except ImportError:
        pass
    try:
        import types

        import antenv
        from trn_agent_boot.trn_boot import _ntff_profile_via_ctypes

        mod = types.ModuleType("antenv.axon_hooks")
        holder = [None]
        mod.set_axon_ntff_profile_hook = lambda h: holder.__setitem__(0, h)
        mod.get_axon_ntff_profile_hook = lambda: holder[0]
        sys.modules["antenv.axon_hooks"] = mod
        antenv.axon_hooks = mod
        mod.set_axon_ntff_profile_hook(
            _ntff_profile_via_ctypes("/opt/axon/libaxon_pjrt.so")
        )
    except Exception:
        pass


def _tiles_for_groups(G):
    """Per-species padded group sizes -> list of (species, slot0, n_atoms)."""
    tiles = []
    slot0 = 0
    for s, g in enumerate(G):
        a = 0
        while a < g:
            na = 4 if g - a >= 4 else g - a
            tiles.append((s, slot0 + a, na))
            a += na
        slot0 += g
    return tiles


def _batches_for_tiles(tiles):
    return [list(range(b, min(b + 4, len(tiles)))) for b in range(0, len(tiles), 4)]


def _build_program(G, S):
    import concourse.bass as bass  # noqa: F401
    import concourse.tile as tile
    from concourse import bacc, mybir

    F32 = mybir.dt.float32
    BF16 = mybir.dt.bfloat16
    X8 = mybir.dt.float8e4 if X_FP8 else BF16
    EXP = mybir.ActivationFunctionType.Exp
    RELU = mybir.ActivationFunctionType.Relu
    MIN = mybir.AluOpType.min
    MAX = mybir.AluOpType.max
    ADD = mybir.AluOpType.add
    SUB = mybir.AluOpType.subtract

    tiles = _tiles_for_groups(G)
    ntiles = len(tiles)
    batches = _batches_for_tiles(tiles)
    nbatches = len(batches)
    WB0 = WPS * N_SPECIES
    CB0 = CPS * N_SPECIES
    WCOLS = WB0 + 128 * nbatches
    CCOLS = CB0 + 2 * nbatches
    # batch k covers consecutive slots [bslot0[k], bslot0[k] + bna[k])
    bslot0 = [tiles[bt[0]][1] for bt in batches]
    bna = [sum(tiles[ti][2] for ti in bt) for bt in batches]
    XB = max(bna) * 384

    nc = bacc.Bacc("TRN2", target_bir_lowering=False, debug=False,
                   num_devices=N_CORES)
    xt = nc.dram_tensor("xt", [128, S * 384], X8, kind="ExternalInput").ap()
    wp = nc.dram_tensor("wp", [128, WCOLS], BF16, kind="ExternalInput").ap()
    cp = nc.dram_tensor("cp", [128, CCOLS], F32, kind="ExternalInput").ap()
    yo = nc.dram_tensor("yo", [1, 512], F32, kind="ExternalOutput").ap()

    with tile.TileContext(nc) as tc:
        with (
            tc.tile_pool(name="wcpool", bufs=1) as wcpool,
            tc.tile_pool(name="xpool", bufs=3) as xpool,
            tc.tile_pool(name="ewpool", bufs=6) as ewpool,
            tc.tile_pool(name="ypool", bufs=6) as ypool,
            tc.tile_pool(name="ppa", bufs=2, space="PSUM") as ppa,
            tc.tile_pool(name="pp1", bufs=2, space="PSUM") as pp1,
            tc.tile_pool(name="ppb0", bufs=1, space="PSUM") as ppb0,
            tc.tile_pool(name="ppb2", bufs=2, space="PSUM") as ppb2,
            tc.tile_pool(name="pp3", bufs=1, space="PSUM") as pp3,
        ):
            # weights/constants on the gpsimd DMA queue, x batches on sync:
            # descriptor generation and transfers overlap. The first species'
            # L0 weight block and the constants go first so the opening
            # matmuls and celus are not gated on the full pack (subtile deps).
            w = wcpool.tile([128, WCOLS], BF16, name="w")
            c = wcpool.tile([128, CCOLS], F32, name="c")
            nc.gpsimd.dma_start(c[:], cp[:])
            s_first = tiles[batches[0][0]][0]
            wA, wB = WPS * s_first, WPS * s_first + WPS
            nc.gpsimd.dma_start(w[:, wA: wA + 480], wp[:, wA: wA + 480])
            nc.gpsimd.dma_start(w[:, wA + 480: wB], wp[:, wA + 480: wB])
            if wA > 0:
                nc.gpsimd.dma_start(w[:, 0:wA], wp[:, 0:wA])
            nc.gpsimd.dma_start(w[:, wB:WCOLS], wp[:, wB:WCOLS])

            def wcol(s, off, n):
                return w[:, s * WPS + off: s * WPS + off + n]

            def ccol(s, k, parts):
                return c[0:parts, s * CPS + k: s * CPS + k + 1]

            p3 = pp3.tile([1, 512], F32)
            # L3 partials accumulate with start=False throughout (the first
            # emitted tile may be a short batch covering only part of the
            # 512 columns), so zero the bank explicitly once.
            nc.vector.memzero(p3[:])
            n3 = [0]

            def celu_v1(y_ap, p_ap, e_ap, t_ap, ebias, mbias):
                # y = (P max mbias) + min(alpha*e^(10P+ebias) - alpha, 0)
                nc.scalar.activation(e_ap, p_ap, EXP, bias=ebias, scale=10.0)
                nc.vector.tensor_scalar(t_ap, e_ap, ALPHA, 0.0, SUB, MIN)
                nc.vector.scalar_tensor_tensor(y_ap, p_ap, mbias, t_ap,
                                               MAX, ADD)

            def celu_v2(y_ap, p_ap, e_ap, t_ap, r_ap, ebias, rbias):
                # y = relu(P + rbias) + min(alpha*e^(10P+ebias) - alpha, 0)
                nc.scalar.activation(e_ap, p_ap, EXP, bias=ebias, scale=10.0)
                nc.scalar.activation(r_ap, p_ap, RELU, bias=rbias, scale=1.0)
                nc.vector.tensor_scalar(t_ap, e_ap, ALPHA, 0.0, SUB, MIN)
                nc.vector.tensor_add(y_ap, r_ap, t_ap)

            xts = {}

            def dma_batch(k, split_first=False):
                t = xpool.tile([128, XB], X8, name="xts")
                o = bslot0[k] * 384
                if split_first:
                    n0 = tiles[batches[k][0]][2] * 384
                    nc.sync.dma_start(t[:, 0:n0], xt[:, o: o + n0])
                    if n0 < bna[k] * 384:
                        nc.sync.dma_start(t[:, n0: bna[k] * 384],
                                          xt[:, o + n0: o + bna[k] * 384])
                else:
                    nc.sync.dma_start(t[:, 0: bna[k] * 384],
                                      xt[:, o: o + bna[k] * 384])
                xts[k] = t

            y0as = {}
            y0bs = {}
            y1s = {}
            y2s = {}
            p0bs = {}

            def l0_mm_thunks(k):
                """Batch k's L0 matmuls as 12 thunks (2 mms each)."""
                batch = batches[k]
                xv = xts[k].rearrange("p (a f m) -> p a f m",
                                      a=XB // 384, f=3, m=128)
                p0b = ppb0.tile([128, 512], F32, tag="pb", name="p0b")
                p0bs[k] = p0b
                p0as = {}
                thunks = []
                for j, ti in enumerate(batch):
                    s, a0, na = tiles[ti]
                    N = na * 128
                    aoff = a0 - bslot0[k]
                    p0a = ppa.tile([128, 512], F32, tag="pa", name="p0a")
                    p0as[ti] = p0a

                    pm = (mybir.MatmulPerfMode.DoublePixel
                          if (L0_DP and X_FP8) else None)

                    def mk(j=j, s=s, na=na, N=N, aoff=aoff, p0a=p0a,
                           pm=pm, fc=0):
                        rhs = xv[:, aoff: aoff + na, fc, :]
                        nc.tensor.matmul(p0a[:, 0:N],
                                         wcol(s, fc * 160, 128),
                                         rhs, start=(fc == 0), stop=(fc == 2),
                                         perf_mode=pm)
                        nc.tensor.matmul(p0b[32 * j: 32 * j + 32, 0:N],
                                         wcol(s, fc * 160 + 128, 32),
                                         rhs, start=(fc == 0), stop=(fc == 2),
                                         tile_position=(0, 32 * j),
                                         perf_mode=pm)
                    for fc in range(3):
                        thunks.append(
                            lambda mk=mk, fc=fc: mk(fc=fc))
                return thunks, p0as

            def l0a_celu(k, j, p0as):
                ti = batches[k][j]
                s, a0, na = tiles[ti]
                N = na * 128
                ea = ewpool.tile([128, 512], BF16, name="ea")
                ta = ewpool.tile([128, 512], BF16, name="ta")
                y0a = ypool.tile([128, 512], BF16, name="y0a")
                celu_v1(y0a[:, 0:N], p0as[ti][:, 0:N], ea[:, 0:N],
                        ta[:, 0:N], ccol(s, 0, 128), ccol(s, 1, 128))
                y0as[ti] = y0a

            def l0b_celu(k):
                L = len(batches[k])
                em = ewpool.tile([128, 512], BF16, name="em")
                tm = ewpool.tile([128, 512], BF16, name="tm")
                y0b = ypool.tile([128, 512], BF16, name="y0b", bufs=2)
                P = 32 * L
                celu_v1(y0b[0:P, :], p0bs[k][0:P, :], em[0:P, :], tm[0:P, :],
                        c[0:P, CB0 + 2 * k: CB0 + 2 * k + 1],
                        c[0:P, CB0 + 2 * k + 1: CB0 + 2 * k + 2])
                y0bs[k] = y0b

            def steady(k):
                """B(k) with A(k+1)'s L0 matmuls interleaved, then C(k)."""
                batch = batches[k]
                y0b = y0bs[k]
                if k + 2 < nbatches:
                    dma_batch(k + 2)
                p1s = {}
                for j, ti in enumerate(batch):
                    s, a0, na = tiles[ti]
                    N = na * 128
                    p1 = pp1.tile([128, 512], F32, tag="p1", name="p1")
                    nc.tensor.matmul(p1[:, 0:N], wcol(s, 480, 128),
                                     y0as[ti][:, 0:N], start=True, stop=False)
                    nc.tensor.matmul(
                        p1[:, 0:N],
                        w[32 * j: 32 * j + 32,
                          WB0 + 128 * k: WB0 + 128 * (k + 1)],
                        y0b[32 * j: 32 * j + 32, 0:N],
                        start=False, stop=True,
                        tile_position=(32 * j, 0))
                    p1s[ti] = p1
                for j, ti in enumerate(batch):
                    s, a0, na = tiles[ti]
                    N = na * 128
                    e1 = ewpool.tile([128, 512], BF16, name="e1")
                    t1 = ewpool.tile([128, 512], BF16, name="t1")
                    r1 = ewpool.tile([128, 512], BF16, name="r1")
                    y1 = ypool.tile([128, 512], BF16, name="y1", bufs=5)
                    celu_v2(y1[:, 0:N], p1s[ti][:, 0:N], e1[:, 0:N],
                            t1[:, 0:N], r1[:, 0:N],
                            ccol(s, 2, 128), ccol(s, 3, 128))
                    y1s[ti] = y1
                if k + 1 < nbatches:
                    thunks, p0as_next = l0_mm_thunks(k + 1)
                else:
                    thunks, p0as_next = [], None
                chunks = [thunks[3 * i: 3 * i + 3] for i in range(4)]
                nnext = len(batches[k + 1]) if p0as_next is not None else 0
                for j, ti in enumerate(batch):
                    for th in chunks[j]:
                        th()
                    if p0as_next is not None and j < nnext:
                        l0a_celu(k + 1, j, p0as_next)
                    if j == len(batch) - 1:
                        # all of next batch's L0 matmuls are in by now:
                        # finish its celus so the merged-L0b stt lands well
                        # before the next cycle's L1b matmuls need it.
                        for ch in chunks[len(batch):]:
                            for th in ch:
                                th()
                        if p0as_next is not None:
                            for jj in range(len(batch), nnext):
                                l0a_celu(k + 1, jj, p0as_next)
                            l0b_celu(k + 1)
                    s, a0, na = tiles[ti]
                    N = na * 128
                    p2 = ppb2.tile([96, 512], F32, tag="p2", name="p2")
                    nc.tensor.matmul(p2[:, 0:N], wcol(s, 608, 96),
                                     y1s[ti][:, 0:N], start=True, stop=True)
                    e2 = ewpool.tile([96, 512], BF16, name="e2")
                    t2 = ewpool.tile([96, 512], BF16, name="t2")
                    y2 = ypool.tile([96, 512], BF16, name="y2")
                    celu_v1(y2[:, 0:N], p2[:, 0:N], e2[:, 0:N], t2[:, 0:N],
                            ccol(s, 4, 96), ccol(s, 5, 96))
                    y2s[ti] = y2
                for j, ti in enumerate(batch):
                    s, a0, na = tiles[ti]
                    N = na * 128
                    nc.tensor.matmul(p3[0:1, 0:N],
                                     wcol(s, 704, 1)[0:96, :],
                                     y2s[ti][0:96, 0:N],
                                     start=False,
                                     stop=(n3[0] == ntiles - 1),
                                     skip_group_check=True)
                    n3[0] += 1

            dma_batch(0, split_first=True)
            if nbatches > 1:
                dma_batch(1)
            thunks0, p0as0 = l0_mm_thunks(0)
            for th in thunks0:
                th()
            for j in range(len(batches[0])):
                l0a_celu(0, j, p0as0)
            l0b_celu(0)
            for k in range(nbatches):
                steady(k)

            t3 = wcpool.tile([1, 512], F32, name="t3")
            nc.scalar.copy(t3[:], p3[:])
            nc.sync.dma_start(yo[:], t3[:])

    nc.compile()
    return nc


def _celu64(z):
    return np.where(z > 0, z, ALPHA * np.expm1(np.minimum(z, 0) / ALPHA))


def _bf16_round(x):
    import ml_dtypes
    return np.asarray(x, np.float32).astype(ml_dtypes.bfloat16).astype(np.float64)


def kernel(fullaev, species, W0, b0, W1, b1, W2, b2, W3, b3):
    import ml_dtypes
    from concourse import bass_utils, mybir

    fullaev = np.ascontiguousarray(np.asarray(fullaev, dtype=np.float32))
    species = np.asarray(species, dtype=np.int32)
    Ws = [np.asarray(w, dtype=np.float32) for w in (W0, W1, W2, W3)]
    bs = [np.asarray(b, dtype=np.float32) for b in (b0, b1, b2, b3)]

    # --- species grouping: per-core slot assignment ---------------------
    ids = [np.where(species == s)[0] for s in range(N_SPECIES)]
    n = [len(i) for i in ids]
    G = [-(-n[s] // N_CORES) if n[s] else 0 for s in range(N_SPECIES)]
    S = sum(G)
    key = (tuple(G), X_FP8, L0_DP)
    if key not in _progs:
        _progs[key] = _build_program(G, S)
    nc = _progs[key]

    tiles = _tiles_for_groups(G)
    batches = _batches_for_tiles(tiles)
    nbatches = len(batches)
    WB0 = WPS * N_SPECIES
    CB0 = CPS * N_SPECIES
    WCOLS = WB0 + 128 * nbatches
    CCOLS = CB0 + 2 * nbatches

    # --- fold constants (float64, with bf16-rounded weights) ------------
    cpack = np.zeros((128, CCOLS), np.float32)
    wpack = np.zeros((128, WCOLS), np.float32)
    c3 = np.zeros(N_SPECIES)
    K0 = np.zeros(N_SPECIES)
    c1s = {}
    for s in range(N_SPECIES):
        w1, w2, w3 = (_bf16_round(Ws[l][s]) for l in (1, 2, 3))
        bb0, bb1, bb2, bb3 = (b[s].astype(np.float64) for b in bs)
        c1 = bb1 + w1 @ bb0
        c1s[s] = c1
        c3[s] = bb3[0] + w3[0] @ bb2
        # device contribution of a dummy (zero-AEV) atom, excluding c3
        y0d = _celu64(bb0) - bb0
        y1d = _celu64(w1 @ y0d + c1)
        y2d = _celu64(w2 @ y1d + bb2) - bb2
        K0[s] = w3[0] @ y2d

        cb = s * CPS
        cpack[:, cb + 0] = 10.0 * bb0[:128] + LNA
        cpack[:, cb + 1] = -bb0[:128]
        cpack[:, cb + 2] = 10.0 * c1 + LNA
        cpack[:, cb + 3] = c1
        cpack[:96, cb + 4] = 10.0 * bb2 + LNA
        cpack[:96, cb + 5] = -bb2

        wb = s * WPS
        for fc in range(3):
            blk = Ws[0][s][:, fc * 128:(fc + 1) * 128].T  # [128in, 160out]
            wpack[:, wb + fc * 160: wb + fc * 160 + 160] = blk
        wpack[:, wb + 480: wb + 608] = Ws[1][s][:, :128].T
        wpack[:, wb + 608: wb + 704] = Ws[2][s].T
        wpack[:96, wb + 704] = Ws[3][s][0, :]

    for bi, batch in enumerate(batches):
        for j, ti in enumerate(batch):
            s = tiles[ti][0]
            b0b = bs[0][s].astype(np.float64)[128:]
            cpack[32 * j: 32 * j + 32, CB0 + 2 * bi] = 10.0 * b0b + LNA
            cpack[32 * j: 32 * j + 32, CB0 + 2 * bi + 1] = -b0b
            wpack[32 * j: 32 * j + 32,
                  WB0 + 128 * bi: WB0 + 128 * (bi + 1)] = Ws[1][s][:, 128:].T

    wpack_b = wpack.astype(ml_dtypes.bfloat16)
    x_np_dtype = mybir.dt.np(mybir.dt.float8e4 if X_FP8 else mybir.dt.bfloat16)

    # --- per-core transposed, species-sorted AEV blocks -----------------
    in_maps = []
    dummy_counts = np.zeros((N_CORES, N_SPECIES), np.int64)
    for cid in range(N_CORES):
        xtc = np.zeros((128, S, 3, 128), np.float32)
        slot0 = 0
        for s in range(N_SPECIES):
            mine = ids[s][cid::N_CORES]
            nr = len(mine)
            dummy_counts[cid, s] = G[s] - nr
            if nr:
                g = fullaev[:, mine, :]               # [128, nr, 384]
                t = g.transpose(2, 1, 0)              # [384, nr, 128]
                xtc[:, slot0: slot0 + nr, :, :] = (
                    t.reshape(3, 128, nr, 128).transpose(1, 2, 0, 3)
                )
            slot0 += G[s]
        xq = xtc.reshape(128, S * 384).astype(x_np_dtype)
        in_maps.append({"xt": xq, "wp": wpack_b, "cp": cpack})

    if TRACE:
        _maybe_register_ntff_hook()
    res = bass_utils.run_bass_kernel_spmd(
        nc, in_maps, core_ids=list(range(N_CORES)), trace=TRACE
    )
    LAST["exec_time_ns"] = res.exec_time_ns
    LAST["trace"] = res.instructions_and_trace[1] if res.instructions_and_trace else None

    out = np.zeros(128, np.float64)
    for cid in range(N_CORES):
        out += (res.results[cid]["yo"][0].astype(np.float64)
                .reshape(4, 128).sum(axis=0))
    for s in range(N_SPECIES):
        out += n[s] * c3[s] - dummy_counts[:, s].sum() * K0[s]
    return out.astype(np.float32)
